# revision 24
# baseline (speedup 1.0000x reference)
"""Trainium2 Bass kernel for an 8-batch image-conditioned decoder layer.

Strategy: pure data-parallel over the batch — core c computes batch element c
end-to-end (causal self-attention, cross-attention over the image tokens,
both layernorms, vocab projection). No collectives.

All matmuls run in bf16 with fp32 PSUM accumulation.  Weights are pre-cast /
pre-tiled on the host into the exact SBUF layouts the TensorEngine consumes
([128 k_inner, k_outer, n]); the vocab projection is streamed from HBM in
512-column chunks (62 full + one 256-wide tail = exactly 32000 columns).

Host-side input prep (beyond dtype/layout): the embedding gather + positional
encoding (x0 = table[tokens] + pos) is computed on the host, and the layernorm
affine transforms are folded into the downstream weights
(Wq2 <- g1*Wq2, bq2 <- bq2 + b1@Wq2; Wp <- g2*Wp, bp <- bp + b2@Wp), so the
device layernorms emit unscaled normalized activations straight to bf16.

Scheduling choices:
  - a PE warm-up matmul burst keeps the HAM clock gate at 8/8 from the start;
  - Vt is computed one a-tile per AV1 iteration so the PE has ~3.5us of real
    matmul work covering each layernorm's latency;
  - x1T/x2T transposes are placed after the corresponding AV matmul groups,
    and the first vocab chunks (tail + strip 0) interleave with the x2T
    transposes so the LN2 tail never idles the PE;
  - the vocab loop is chunk-major, qt-ascending, at the bf16 roofline.
"""

import os
import sys

for _p in ("/opt/trn_rl_repo", "/root/.axon_site/_ro/trn_rl_repo"):
    if os.path.isdir(_p) and _p not in sys.path:
        sys.path.append(_p)

import numpy as np
import ml_dtypes

BF16 = ml_dtypes.bfloat16

# Problem dims (hardcoded per spec)
V, D, DI, S, B, NI = 32000, 1024, 768, 512, 8, 197
EPS = 1e-5
P = 128
ST = S // P          # 4 seq tiles
DT = D // P          # 8 model-dim tiles
DIT = DI // P        # 6 image-dim tiles
NIT = 2              # image tokens: 197 -> 2 partition tiles (128 + 69)
NI_PAD = 256
CN = 512             # vocab chunk width
NCHUNK = 63          # 62 full chunks + one 256-wide tail = 32000
LAST_W = 256
GRP = 2              # full chunks per output strip
NFULL = 62
NGRP = NFULL // GRP  # 31 full strips; the tail chunk is its own strip
N_CORES = 8
N_WARMUP = 10        # PE warm-up matmuls before the first projections
SCALE = 1.0 / float(np.sqrt(np.float32(D)))

_CACHE = {}
LAST_RESULTS = None


def _build_program():
    import concourse.bacc as bacc
    import concourse.bass as bass
    import concourse.mybir as mybir
    from concourse.masks import make_identity
    from concourse.tile import TileContext

    f32 = mybir.dt.float32
    bf16 = mybir.dt.bfloat16
    X = mybir.AxisListType.X
    ALU = mybir.AluOpType
    ACT_F = mybir.ActivationFunctionType

    nc = bacc.Bacc("TRN2", target_bir_lowering=False, debug=False,
                   num_devices=N_CORES)

    # ---- I/O ----
    h_x0 = nc.dram_tensor("x0", [P, ST, D], bf16, kind="ExternalInput")
    h_x0t = nc.dram_tensor("x0t", [P, DT, S], bf16, kind="ExternalInput")
    h_img = nc.dram_tensor("img_t", [P, DIT, NI], bf16, kind="ExternalInput")
    h_wq1 = nc.dram_tensor("wq1", [P, DT, D], bf16, kind="ExternalInput")
    h_wk1 = nc.dram_tensor("wk1", [P, DT, D], bf16, kind="ExternalInput")
    h_wv1 = nc.dram_tensor("wv1", [P, DT, D], bf16, kind="ExternalInput")
    h_wq2 = nc.dram_tensor("wq2", [P, DT, D], bf16, kind="ExternalInput")
    h_wk2 = nc.dram_tensor("wk2", [P, DIT, D], bf16, kind="ExternalInput")
    h_wv2 = nc.dram_tensor("wv2", [P, DIT, D], bf16, kind="ExternalInput")
    h_wp = nc.dram_tensor("wp", [NCHUNK, P, DT, CN], bf16, kind="ExternalInput")
    h_bq1 = nc.dram_tensor("bq1", [P, DT], f32, kind="ExternalInput")
    h_bk1 = nc.dram_tensor("bk1", [P, DT], f32, kind="ExternalInput")
    h_bq2 = nc.dram_tensor("bq2", [P, DT], f32, kind="ExternalInput")
    h_bk2 = nc.dram_tensor("bk2", [P, DT], f32, kind="ExternalInput")
    h_bv1 = nc.dram_tensor("bv1", [D], bf16, kind="ExternalInput")
    h_bv2 = nc.dram_tensor("bv2", [D], bf16, kind="ExternalInput")
    h_bp = nc.dram_tensor("bp", [V], bf16, kind="ExternalInput")
    h_g1 = nc.dram_tensor("g1", [D], bf16, kind="ExternalInput")
    h_b1 = nc.dram_tensor("b1", [D], bf16, kind="ExternalInput")
    h_out = nc.dram_tensor("out", [S, V], bf16, kind="ExternalOutput")

    def bcast(handle, n, offset=0):
        ap = handle[:]
        return bass.AP(tensor=ap.tensor, offset=offset, ap=[[0, P], [1, n]])

    with TileContext(nc) as tc:
        import contextlib
        ctx = contextlib.ExitStack()
        with ctx:
            const = ctx.enter_context(tc.tile_pool(name="const", bufs=1))
            xb_p = ctx.enter_context(tc.tile_pool(name="xb", bufs=3))
            xt_p = ctx.enter_context(tc.tile_pool(name="xt", bufs=2))
            qk_p = ctx.enter_context(tc.tile_pool(name="qk", bufs=2))
            v_p = ctx.enter_context(tc.tile_pool(name="vp", bufs=2))
            k2t_p = ctx.enter_context(tc.tile_pool(name="k2t", bufs=1))
            pb_p = ctx.enter_context(tc.tile_pool(name="pb", bufs=4))
            pt_p = ctx.enter_context(tc.tile_pool(name="pt", bufs=2))
            msk_p = ctx.enter_context(tc.tile_pool(name="msk", bufs=1))
            xpre_p = ctx.enter_context(tc.tile_pool(name="xpre", bufs=2))
            stat_p = ctx.enter_context(tc.tile_pool(name="stat", bufs=4))
            wts_p = ctx.enter_context(tc.tile_pool(name="wts", bufs=3))
            wp_p = ctx.enter_context(tc.tile_pool(name="wpp", bufs=3))
            bp_p = ctx.enter_context(tc.tile_pool(name="bpp", bufs=2))
            osb_p = ctx.enter_context(tc.tile_pool(name="osb", bufs=8))
            ps = ctx.enter_context(tc.tile_pool(name="ps", bufs=8, space="PSUM"))

            # ---- critical DMAs first: per-k slices interleaved across
            # both hw queues so the k-outer QT can track the stream ----
            x0T = xt_p.tile([P, DT, S], bf16, tag="xt", name="x0t")
            wq1_sb = wts_p.tile([P, DT, D], bf16, tag="wts")
            for k in range(DT):
                e0 = nc.sync if k % 2 == 0 else nc.scalar
                e1 = nc.scalar if k % 2 == 0 else nc.sync
                e0.dma_start(out=x0T[:, k, :], in_=h_x0t[:, k, :])
                e1.dma_start(out=wq1_sb[:, k, :], in_=h_wq1[:, k, :])
            wk1_sb = wts_p.tile([P, DT, D], bf16, tag="wts")
            for k in range(DT):
                eng = nc.sync if k % 2 == 0 else nc.scalar
                eng.dma_start(out=wk1_sb[:, k, :], in_=h_wk1[:, k, :])
            bq1s = const.tile([P, DT], f32)
            bk1s = const.tile([P, DT], f32)
            bq2s = const.tile([P, DT], f32)
            bk2s = const.tile([P, DT], f32)
            for t, h in ((bq1s, h_bq1), (bk1s, h_bk1), (bq2s, h_bq2),
                         (bk2s, h_bk2)):
                nc.scalar.dma_start(out=t, in_=h[:])

            # ---- PE warm-up fodder (no data deps beyond one memset) ----
            zeros = const.tile([P, CN], bf16)
            nc.vector.memset(zeros, 0.0)
            ident = const.tile([P, P], bf16)
            make_identity(nc, ident)
            ps_warm = ps.tile([P, CN], f32, tag="ps", name="warm")

            def warm_mm(n=1):
                for _ in range(n):
                    nc.tensor.matmul(ps_warm, lhsT=zeros[:, :P], rhs=zeros,
                                     start=True, stop=True)

            warm_mm(N_WARMUP)

            # ---- PE-transpose helper: [P, ST, D] seq-part -> [P, DT, S];
            # optionally interleave warm-up matmuls so the HAM clock gate
            # stays at 8/8 (transpose-mode does not count as PE activity) ----
            def psum_copy(out, in_, use_act):
                if use_act:
                    nc.scalar.activation(out=out, in_=in_, func=ACT_F.Copy)
                else:
                    nc.vector.tensor_copy(out=out, in_=in_)

            def transpose_x(xb_tile, xt_tile, a_list=range(ST), warm=0):
                for a in a_list:
                    for db in range(DT):
                        tp = ps.tile([P, CN], bf16, tag="ps", name="tp")
                        nc.tensor.transpose(
                            out=tp[:, :P],
                            in_=xb_tile[:, a, db * P:(db + 1) * P],
                            identity=ident)
                        psum_copy(xt_tile[:, db, a * P:(a + 1) * P],
                                  tp[:, :P], db % 2 == 0)
                    if warm:
                        warm_mm(warm)

            # ---- remaining weight/bias DMAs in need order ----
            wk2_sb = wts_p.tile([P, DIT, D], bf16, tag="wts")
            nc.sync.dma_start(out=wk2_sb, in_=h_wk2[:])
            x0b = xb_p.tile([P, ST, D], bf16, tag="xb", name="x0b")
            nc.sync.dma_start(out=x0b, in_=h_x0[:])
            img_sb = const.tile([P, DIT, NI], bf16)
            nc.scalar.dma_start(out=img_sb, in_=h_img[:])
            wv1_sb = wts_p.tile([P, DT, D], bf16, tag="wts")
            nc.scalar.dma_start(out=wv1_sb, in_=h_wv1[:])

            epst = const.tile([P, 1], f32)
            nc.vector.memset(epst, EPS)
            trimask = const.tile([P, P], f32)
            nc.gpsimd.memset(trimask, 0.0)
            nc.gpsimd.affine_select(
                out=trimask, in_=trimask, compare_op=ALU.is_ge, fill=-1e10,
                base=0, pattern=[[-1, P]], channel_multiplier=1)

            g1b = const.tile([P, D], bf16)
            b1b = const.tile([P, D], bf16)
            bv1b = const.tile([P, D], bf16)
            bv2b = const.tile([P, D], bf16)
            for i, (t, h) in enumerate(((bv1b, h_bv1), (g1b, h_g1),
                                        (b1b, h_b1), (bv2b, h_bv2))):
                eng = nc.scalar if i % 2 == 0 else nc.sync
                eng.dma_start(out=t, in_=bcast(h, D))

            # late-needed weights last on each queue
            wq2_sb = wts_p.tile([P, DT, D], bf16, tag="wts")
            nc.sync.dma_start(out=wq2_sb, in_=h_wq2[:])
            wv2_sb = wts_p.tile([P, DIT, D], bf16, tag="wts")
            nc.scalar.dma_start(out=wv2_sb, in_=h_wv2[:])

            # ---- projections ----
            def proj_T(w_sb, b_sb, rhsT, name):
                """QT/KT-style: out[P, DT, S] bf16 = (W.T @ x.T) + b, d-part."""
                o = qk_p.tile([P, DT, S], bf16, tag="qk", name=name)
                for m in range(DT):
                    pm = ps.tile([P, CN], f32, tag="ps", name="pm")
                    for k in range(DT):
                        nc.tensor.matmul(pm, lhsT=w_sb[:, k, m * P:(m + 1) * P],
                                         rhs=rhsT[:, k, :],
                                         start=(k == 0), stop=(k == DT - 1))
                    nc.scalar.activation(out=o[:, m, :], in_=pm,
                                         func=ACT_F.Identity,
                                         bias=b_sb[:, m:m + 1], scale=1.0)
                return o

            def proj_T_kouter(w_sb, b_sb, rhsT, name):
                """k-outer variant using all 8 PSUM banks: each k-slice of
                weights/activations is consumed as soon as its DMA lands."""
                o = qk_p.tile([P, DT, S], bf16, tag="qk", name=name)
                pms = [ps.tile([P, CN], f32, tag="ps", name=f"pm{name}{m}")
                       for m in range(DT)]
                for k in range(DT):
                    for m in range(DT):
                        nc.tensor.matmul(pms[m],
                                         lhsT=w_sb[:, k, m * P:(m + 1) * P],
                                         rhs=rhsT[:, k, :],
                                         start=(k == 0), stop=(k == DT - 1))
                for m in range(DT):
                    nc.scalar.activation(out=o[:, m, :], in_=pms[m],
                                         func=ACT_F.Identity,
                                         bias=b_sb[:, m:m + 1], scale=1.0)
                return o

            QT = proj_T_kouter(wq1_sb, bq1s, x0T, "qt")
            KT = proj_T_kouter(wk1_sb, bk1s, x0T, "kt")

            # ---- causal scores + softmax (all qt up front) ----
            Pbs = []
            rinv1 = stat_p.tile([P, ST], f32, tag="rinv")
            for qt in range(ST):
                width = (qt + 1) * P
                pm = ps.tile([P, CN], f32, tag="ps")
                for k in range(DT):
                    nc.tensor.matmul(pm[:, :width],
                                     lhsT=QT[:, k, qt * P:(qt + 1) * P],
                                     rhs=KT[:, k, :width],
                                     start=(k == 0), stop=(k == DT - 1))
                masked = msk_p.tile([P, CN], f32, tag="msk")
                if qt > 0:
                    nc.vector.tensor_copy(out=masked[:, :qt * P],
                                          in_=pm[:, :qt * P])
                nc.vector.tensor_tensor(out=masked[:, qt * P:width],
                                        in0=pm[:, qt * P:width], in1=trimask,
                                        op=ALU.add)
                nmax = stat_p.tile([P, 1], f32, tag="nmax")
                nc.vector.reduce_max(nmax, masked[:, :width], axis=X,
                                     negate=True)
                Pb = pb_p.tile([P, CN], bf16, tag="pb", name=f"pb{qt}")
                rsum = stat_p.tile([P, 1], f32, tag="rsum")
                nc.scalar.activation(out=Pb[:, :width], in_=masked[:, :width],
                                     func=ACT_F.Exp, bias=nmax, scale=1.0,
                                     accum_out=rsum)
                nc.vector.reciprocal(out=rinv1[:, qt:qt + 1], in_=rsum)
                Pbs.append(Pb)

            # ---- K2T fills the first softmax window ----
            K2T = k2t_p.tile([P, DT, NI_PAD], bf16, tag="k2t")
            for m in range(DT):
                pm = ps.tile([P, CN], f32, tag="ps")
                for k in range(DIT):
                    nc.tensor.matmul(pm[:, :NI],
                                     lhsT=wk2_sb[:, k, m * P:(m + 1) * P],
                                     rhs=img_sb[:, k, :],
                                     start=(k == 0), stop=(k == DIT - 1))
                nc.scalar.activation(out=K2T[:, m, :NI], in_=pm[:, :NI],
                                     func=ACT_F.Identity,
                                     bias=bk2s[:, m:m + 1], scale=1.0)

            def layernorm(xpre, out_sl):
                """xpre [P, D] f32 -> out_sl [P, D] bf16 UNSCALED normalized
                activations (gamma/beta are folded into downstream weights)."""
                stats = stat_p.tile([P, 2, 6], f32, tag="bnst")
                for sg in range(2):
                    nc.vector.bn_stats(out=stats[:, sg, :],
                                       in_=xpre[:, sg * CN:(sg + 1) * CN])
                mv = stat_p.tile([P, 2], f32, tag="bnmv")
                nc.vector.bn_aggr(out=mv, in_=stats)
                rstd = stat_p.tile([P, 1], f32, tag="rstd")
                nc.scalar.activation(out=rstd, in_=mv[:, 1:2], func=ACT_F.Sqrt,
                                     bias=epst, scale=1.0)
                nc.vector.reciprocal(out=rstd, in_=rstd)
                nmr = stat_p.tile([P, 1], f32, tag="nmr")
                nc.vector.scalar_tensor_tensor(out=nmr, in0=mv[:, 0:1],
                                               scalar=-1.0, in1=rstd,
                                               op0=ALU.mult, op1=ALU.mult)
                nc.scalar.activation(out=out_sl, in_=xpre, func=ACT_F.Identity,
                                     bias=nmr, scale=rstd)

            # ---- AV1 + residual + LN1, with Vt interleaved per qt so the
            # PE has real matmul work covering each layernorm's latency ----
            PT = pt_p.tile([P, ST, S], bf16, tag="pt")
            Vt = v_p.tile([P, ST, D], bf16, tag="v")
            xn1 = xb_p.tile([P, ST, D], bf16, tag="xb", name="xn1")
            x1b = xb_p.tile([P, ST, D], bf16, tag="xb", name="x1b")
            for qt in range(ST):
                # Vt a-tile qt (needed by AV1 kt<=qt, so always ready)
                for nh in range(2):
                    pm = ps.tile([P, CN], f32, tag="ps")
                    for k in range(DT):
                        nc.tensor.matmul(
                            pm, lhsT=x0T[:, k, qt * P:(qt + 1) * P],
                            rhs=wv1_sb[:, k, nh * CN:(nh + 1) * CN],
                            start=(k == 0), stop=(k == DT - 1))
                    nc.vector.tensor_tensor(
                        out=Vt[:, qt, nh * CN:(nh + 1) * CN], in0=pm,
                        in1=bv1b[:, nh * CN:(nh + 1) * CN], op=ALU.add)
                for kt in range(qt + 1):
                    tp = ps.tile([P, CN], bf16, tag="ps", name="tp")
                    nc.tensor.transpose(out=tp[:, :P],
                                        in_=Pbs[qt][:, kt * P:(kt + 1) * P],
                                        identity=ident)
                    psum_copy(PT[:, kt, qt * P:(qt + 1) * P], tp[:, :P],
                              kt % 2 == 0)
                xpre = xpre_p.tile([P, D], f32, tag="xpre")
                for nh in range(2):
                    pm = ps.tile([P, CN], f32, tag="ps")
                    for kt in range(qt + 1):
                        nc.tensor.matmul(pm, lhsT=PT[:, kt, qt * P:(qt + 1) * P],
                                         rhs=Vt[:, kt, nh * CN:(nh + 1) * CN],
                                         start=(kt == 0), stop=(kt == qt))
                    nc.vector.scalar_tensor_tensor(
                        out=xpre[:, nh * CN:(nh + 1) * CN], in0=pm,
                        scalar=rinv1[:, qt:qt + 1],
                        in1=x0b[:, qt, nh * CN:(nh + 1) * CN],
                        op0=ALU.mult, op1=ALU.add)
                layernorm(xpre, xn1[:, qt, :])
                # x1 (with LN1 affine) is only needed for the AV2 residual;
                # keep it off the x1T critical path
                nc.vector.tensor_tensor(out=x1b[:, qt, :], in0=xn1[:, qt, :],
                                        in1=g1b, op=ALU.mult)
                nc.vector.tensor_tensor(out=x1b[:, qt, :], in0=x1b[:, qt, :],
                                        in1=b1b, op=ALU.add)

            # ---- V2t fills the LN1 tail before x1T / Q2T need it ----
            V2t = v_p.tile([P, NIT, D], bf16, tag="v")
            nc.vector.memset(V2t, 0.0)
            for a in range(NIT):
                pa = P if a == 0 else NI - P
                for nh in range(2):
                    pm = ps.tile([P, CN], f32, tag="ps")
                    for k in range(DIT):
                        nc.tensor.matmul(
                            pm[:pa, :], lhsT=img_sb[:, k, a * P:a * P + pa],
                            rhs=wv2_sb[:, k, nh * CN:(nh + 1) * CN],
                            start=(k == 0), stop=(k == DIT - 1))
                    nc.vector.tensor_tensor(
                        out=V2t[:pa, a, nh * CN:(nh + 1) * CN], in0=pm[:pa, :],
                        in1=bv2b[:pa, nh * CN:(nh + 1) * CN], op=ALU.add)

            x1T = xt_p.tile([P, DT, S], bf16, tag="xt", name="x1t")
            transpose_x(xn1, x1T, warm=1)

            Q2T = proj_T(wq2_sb, bq2s, x1T, "q2t")

            # ---- cross-attn scores + softmax (all qt up front) ----
            P2bs = []
            rinv2 = stat_p.tile([P, ST], f32, tag="rinv")
            for qt in range(ST):
                pm = ps.tile([P, CN], f32, tag="ps")
                for k in range(DT):
                    nc.tensor.matmul(pm[:, :NI],
                                     lhsT=Q2T[:, k, qt * P:(qt + 1) * P],
                                     rhs=K2T[:, k, :NI],
                                     start=(k == 0), stop=(k == DT - 1))
                nmax = stat_p.tile([P, 1], f32, tag="nmax")
                nc.vector.reduce_max(nmax, pm[:, :NI], axis=X, negate=True)
                P2b = pb_p.tile([P, NI_PAD], bf16, tag="pb", name=f"p2b{qt}")
                nc.vector.memset(P2b[:, NI:], 0.0)
                rsum = stat_p.tile([P, 1], f32, tag="rsum")
                nc.scalar.activation(out=P2b[:, :NI], in_=pm[:, :NI],
                                     func=ACT_F.Exp, bias=nmax, scale=1.0,
                                     accum_out=rsum)
                nc.vector.reciprocal(out=rinv2[:, qt:qt + 1], in_=rsum)
                P2bs.append(P2b)

            # ---- AV2 + residual + LN2 ----
            PT2 = pt_p.tile([P, NIT, S], bf16, tag="pt")
            xn2 = xb_p.tile([P, ST, D], bf16, tag="xb", name="xn2")
            for qt in range(ST):
                for kt in range(NIT):
                    tp = ps.tile([P, CN], bf16, tag="ps", name="tp")
                    nc.tensor.transpose(out=tp[:, :P],
                                        in_=P2bs[qt][:, kt * P:(kt + 1) * P],
                                        identity=ident)
                    psum_copy(PT2[:, kt, qt * P:(qt + 1) * P], tp[:, :P],
                              kt % 2 == 0)
                xpre = xpre_p.tile([P, D], f32, tag="xpre")
                for nh in range(2):
                    pm = ps.tile([P, CN], f32, tag="ps")
                    for kt in range(NIT):
                        nc.tensor.matmul(pm, lhsT=PT2[:, kt, qt * P:(qt + 1) * P],
                                         rhs=V2t[:, kt, nh * CN:(nh + 1) * CN],
                                         start=(kt == 0), stop=(kt == NIT - 1))
                    nc.vector.scalar_tensor_tensor(
                        out=xpre[:, nh * CN:(nh + 1) * CN], in0=pm,
                        scalar=rinv2[:, qt:qt + 1],
                        in1=x1b[:, qt, nh * CN:(nh + 1) * CN],
                        op0=ALU.mult, op1=ALU.add)
                layernorm(xpre, xn2[:, qt, :])

            # ---- vocab projection, streamed in CN-column chunks.
            # x2T transposes interleave with the first vocab units so the
            # LN2 tails are covered by real matmuls. ----
            x2T = xt_p.tile([P, DT, S], bf16, tag="xt", name="x2t")

            def vocab_unit(wp_sb, osb, qt, col0, w, bp_tile):
                """8 bf16 matmuls + bias add into osb[qt][col0:col0+w]."""
                pm = ps.tile([P, CN], f32, tag="ps")
                for k in range(DT):
                    nc.tensor.matmul(
                        pm[:, :w], lhsT=x2T[:, k, qt * P:(qt + 1) * P],
                        rhs=wp_sb[:, k, :w],
                        start=(k == 0), stop=(k == DT - 1))
                nc.vector.tensor_tensor(
                    out=osb[qt][:, col0:col0 + w], in0=pm[:, :w],
                    in1=bp_tile[:, col0:col0 + w], op=ALU.add)

            # prefetch tail + strip-0 weights and biases
            bp_t = bp_p.tile([P, GRP * CN], bf16, tag="bp")
            nc.scalar.dma_start(out=bp_t[:, :LAST_W],
                                in_=bcast(h_bp, LAST_W, offset=NFULL * CN))
            wp_t_sb = wp_p.tile([P, DT, CN], bf16, tag="wp")
            nc.sync.dma_start(out=wp_t_sb[:, :, :LAST_W],
                              in_=h_wp[NFULL][:, :, :LAST_W])
            osb_t = [osb_p.tile([P, GRP * CN], bf16, tag="osb",
                                name=f"osb_t_{q}") for q in range(ST)]
            bp_0 = bp_p.tile([P, GRP * CN], bf16, tag="bp")
            nc.scalar.dma_start(out=bp_0, in_=bcast(h_bp, GRP * CN, offset=0))
            wp_c0 = wp_p.tile([P, DT, CN], bf16, tag="wp")
            nc.sync.dma_start(out=wp_c0, in_=h_wp[0])
            wp_c1 = wp_p.tile([P, DT, CN], bf16, tag="wp")
            nc.scalar.dma_start(out=wp_c1, in_=h_wp[1])
            osb_0 = [osb_p.tile([P, GRP * CN], bf16, tag="osb",
                                name=f"osb_0_{q}") for q in range(ST)]

            transpose_x(xn2, x2T, a_list=[0], warm=1)
            transpose_x(xn2, x2T, a_list=[1], warm=1)
            vocab_unit(wp_t_sb, osb_t, 0, 0, LAST_W, bp_t)
            transpose_x(xn2, x2T, a_list=[2], warm=1)
            vocab_unit(wp_c0, osb_0, 0, 0, CN, bp_0)
            vocab_unit(wp_t_sb, osb_t, 1, 0, LAST_W, bp_t)
            transpose_x(xn2, x2T, a_list=[3], warm=1)
            vocab_unit(wp_c0, osb_0, 1, 0, CN, bp_0)
            vocab_unit(wp_t_sb, osb_t, 2, 0, LAST_W, bp_t)
            vocab_unit(wp_t_sb, osb_t, 3, 0, LAST_W, bp_t)
            for qt in range(ST):
                out_eng = nc.sync if qt < 2 else nc.scalar
                out_eng.dma_start(
                    out=h_out[qt * P:(qt + 1) * P, NFULL * CN:V],
                    in_=osb_t[qt][:, :LAST_W])
            for qt in range(2, ST):
                vocab_unit(wp_c0, osb_0, qt, 0, CN, bp_0)
            for qt in range(ST):
                vocab_unit(wp_c1, osb_0, qt, CN, CN, bp_0)
            for qt in range(ST):
                out_eng = nc.sync if qt < 2 else nc.scalar
                out_eng.dma_start(
                    out=h_out[qt * P:(qt + 1) * P, 0:GRP * CN],
                    in_=osb_0[qt])

            for g in range(1, NGRP):
                bp_bc = bp_p.tile([P, GRP * CN], bf16, tag="bp")
                nc.scalar.dma_start(out=bp_bc,
                                    in_=bcast(h_bp, GRP * CN,
                                              offset=g * GRP * CN))
                osb = [osb_p.tile([P, GRP * CN], bf16, tag="osb",
                                  name=f"osb_{g}_{q}")
                       for q in range(ST)]
                for cc in range(GRP):
                    c = g * GRP + cc
                    wp_sb = wp_p.tile([P, DT, CN], bf16, tag="wp")
                    dma_eng = nc.sync if c % 2 == 0 else nc.scalar
                    dma_eng.dma_start(out=wp_sb, in_=h_wp[c])
                    for qt in range(ST):
                        vocab_unit(wp_sb, osb, qt, cc * CN, CN, bp_bc)
                    if g == NGRP - 1:
                        for qt in range(ST):
                            out_eng = nc.sync if qt < 2 else nc.scalar
                            out_eng.dma_start(
                                out=h_out[qt * P:(qt + 1) * P,
                                          c * CN:(c + 1) * CN],
                                in_=osb[qt][:, cc * CN:(cc + 1) * CN])
                if g < NGRP - 1:
                    for qt in range(ST):
                        out_eng = nc.sync if qt < 2 else nc.scalar
                        out_eng.dma_start(
                            out=h_out[qt * P:(qt + 1) * P,
                                      g * GRP * CN:(g + 1) * GRP * CN],
                            in_=osb[qt])

    nc.compile()
    return nc


def _tile_sq(w, kt):
    """[K, N] -> [128, K//128, N] contiguous."""
    k, n = w.shape
    assert k == kt * P
    return np.ascontiguousarray(
        w.reshape(kt, P, n).transpose(1, 0, 2)).astype(BF16)


def _prep_inputs(inputs):
    g = lambda name: np.asarray(inputs[name], dtype=np.float32)
    tokens = np.asarray(inputs["tokens"]).astype(np.int64)
    img = g("img_emb")
    table = g("emb_table")

    # positional encoding (same closed form as the model definition)
    posn = np.arange(S)[:, None].astype(np.float32)
    i = np.arange(0, D, 2).astype(np.float32)
    ang = posn / np.power(10000.0, i / D)
    pos = np.zeros((S, D), dtype=np.float32)
    pos[:, 0::2] = np.sin(ang)
    pos[:, 1::2] = np.cos(ang)

    # fold the LN affine transforms into the downstream weights:
    #   x1 @ Wq2 + bq2 = xn1 @ (g1*Wq2) + (bq2 + b1@Wq2)
    #   x2 @ Wp  + bp  = xn2 @ (g2*Wp)  + (bp  + b2@Wp)
    g1 = g("g1"); b1 = g("b1"); g2 = g("g2"); b2 = g("b2")
    wq2 = g("Wq2") * g1[:, None]
    bq2 = g("bq2") + b1 @ g("Wq2")
    wp = g("Wp") * g2[:, None]
    bp = (g("bp") + b2 @ g("Wp")).astype(BF16)

    wp_pad = np.zeros((D, NCHUNK * CN), dtype=np.float32)
    wp_pad[:, :V] = wp
    wp_t = np.ascontiguousarray(
        wp_pad.reshape(DT, P, NCHUNK, CN).transpose(2, 1, 0, 3)).astype(BF16)

    def bias_tiled(b):
        return np.ascontiguousarray(b.reshape(DT, P).T).astype(np.float32)

    shared = {
        "wq1": _tile_sq(g("Wq1") * SCALE, DT),
        "wk1": _tile_sq(g("Wk1"), DT),
        "wv1": _tile_sq(g("Wv1"), DT),
        "wq2": _tile_sq(wq2 * SCALE, DT),
        "wk2": _tile_sq(g("Wk2"), DIT),
        "wv2": _tile_sq(g("Wv2"), DIT),
        "wp": wp_t,
        "bq1": bias_tiled(g("bq1") * SCALE),
        "bk1": bias_tiled(g("bk1")),
        "bq2": bias_tiled(bq2 * SCALE),
        "bk2": bias_tiled(g("bk2")),
        "bv1": g("bv1").astype(BF16),
        "bv2": g("bv2").astype(BF16),
        "bp": bp,
        "g1": g1.astype(BF16), "b1": b1.astype(BF16),
    }
    in_maps = []
    for c in range(N_CORES):
        m = dict(shared)
        # embedding gather + positional encoding on the host, pre-tiled
        x0 = table[tokens[c]] + pos  # [S, D] f32
        m["x0"] = np.ascontiguousarray(
            x0.reshape(ST, P, D).transpose(1, 0, 2)).astype(BF16)
        m["x0t"] = np.ascontiguousarray(
            x0.T.reshape(DT, P, S).transpose(1, 0, 2)).astype(BF16)
        m["img_t"] = np.ascontiguousarray(
            img[c].T.reshape(DIT, P, NI).transpose(1, 0, 2)).astype(BF16)
        in_maps.append(m)
    return in_maps


def _ensure_axon_hooks():
    """bass_utils imports antenv.axon_hooks when BASS_TRACE is set; stub it
    if the module is absent so tracing degrades instead of crashing."""
    try:
        import antenv.axon_hooks  # noqa: F401
    except ImportError:
        import types
        mod = types.ModuleType("antenv.axon_hooks")
        mod.get_axon_ntff_profile_hook = lambda: None
        mod.set_axon_ntff_profile_hook = lambda h: None
        sys.modules["antenv.axon_hooks"] = mod


def kernel(**inputs):
    global LAST_RESULTS
    _ensure_axon_hooks()
    from concourse.bass_utils import run_bass_kernel_spmd

    if "nc" not in _CACHE:
        _CACHE["nc"] = _build_program()
    nc = _CACHE["nc"]

    in_maps = _prep_inputs(inputs)
    res = run_bass_kernel_spmd(nc, in_maps, core_ids=list(range(N_CORES)))
    LAST_RESULTS = res
    out = np.stack([res.results[c]["out"].astype(np.float32)
                    for c in range(N_CORES)])
    return out
